# revision 43
# baseline (speedup 1.0000x reference)
"""Bahdanau-style attention kernel for Trainium2 (8 NeuronCores, SPMD).

Math (per batch row b):
    h_proj = hidden @ a_w[:DEC]                       (DEC,)
    e_proj[s, :] = enc[s, :] @ a_w[DEC:]              (S, DEC)
    energy = tanh(e_proj + h_proj + a_b)              (S, DEC)
    scores = energy @ v_w                             (S,)
    scores = where(mask == 0, -1e10, scores)
    attn = softmax(scores)                            (S,)
    out = attn @ enc                                  (ENC,)

Sharding: data-parallel over batch (32 rows -> 4 rows on each of 8 cores);
weights replicated (pre-quantized to fp8*64 on host).

Per-core strategy:
  - The weighted sum runs in bf16 from natural-layout [tok, e] chunks
    (host-cast enc); softmax-averaging keeps per-element quantization
    error in the output, so fp8 enc there would blow the 2e-2 gate.
  - e_proj runs in fp8 with MatmulPerfMode.DoubleRow (2 k-tiles per
    instruction at 0.5 cycles/row). The transposed fp8 operand comes from
    the xbar DMA transpose moving fp8 PAIRS as uint16 lanes straight from
    DRAM: out[p, g, q](u16) = enc-pair(e=2(128g+p)(+0/1), tok q). The pair
    interleave is absorbed by the DoubleRow k-pair dimension with a
    host-permuted weight layout w8[p, g, i, d] = 64*a_w[DEC+2(128g+p)+i, d],
    so no on-chip bf16->fp8 cast and no bf16 staging is needed.
  - e_proj PSUM is [128d, 2x512tok] (a chunk pair, 2 banks) so one tanh
    activation covers 1024 tokens per d-tile, amortizing the ~185ns
    ScalarE access overhead; bias (h_proj + a_b) is per-partition.
  - scores = v . tanh as fp8 DoubleRow over d-tile pairs (v padded to
    M=2 / k-stride 512 for the dual-fp8 ldweights ISA restriction); exp
    is fused into the PSUM evacuation (scale=1/64 undoes the *64 weight
    scaling). Softmax tail on DVE.
  - The weighted sum accumulates chunk-major into one PSUM bank
    (e-halves at partitions 0/32) so nat buffers free chunk-by-chunk,
    and each iteration emits weighted(b-1) before eproj(b) so next-row
    DMA overlaps this row's PE work.
"""

import numpy as np
from contextlib import ExitStack

B, S, ENC, DEC = 32, 2048, 1024, 1024
N_CORES = 8
BC = B // N_CORES  # batch rows per core
CH = 512           # tokens per chunk


def build_bass_kernel(bc=BC, s=S, e_dim=ENC, d_dim=DEC, debug=False):
    import concourse.bass as bass
    import concourse.tile as tile
    from concourse import bacc, mybir

    f32 = mybir.dt.float32
    bf16 = mybir.dt.bfloat16
    fp8 = mybir.dt.float8e4
    u16 = mybir.dt.uint16
    i32 = mybir.dt.int32
    Tanh = mybir.ActivationFunctionType.Tanh
    Exp = mybir.ActivationFunctionType.Exp
    DR = mybir.MatmulPerfMode.DoubleRow

    assert s % (2 * CH) == 0 and e_dim % 256 == 0 and d_dim % 256 == 0
    n_chunks = s // CH             # 512-token chunks per batch row
    n_st = CH // 128               # s-tiles per chunk
    n_g = e_dim // 256             # e pair-groups (256 e-rows per group)
    n_dt = d_dim // 128            # d (output) tiles for e_proj
    n_ec = e_dim // 512            # 512-wide e chunks for the weighted sum
    n_ct = s // 128                # s-tiles per row

    nc = bacc.Bacc("TRN2", target_bir_lowering=False, debug=debug)

    ench_h = nc.dram_tensor("ench", [bc, s, e_dim], fp8, kind="ExternalInput")
    encl_h = nc.dram_tensor("encl", [bc, s, e_dim], fp8, kind="ExternalInput")
    mskT_h = nc.dram_tensor("maskT", [bc, 128, s // 128], bf16, kind="ExternalInput")
    w8_h = nc.dram_tensor("w8", [128, n_g, 2, d_dim], fp8, kind="ExternalInput")
    wd8_h = nc.dram_tensor("wd8", [128, n_dt, d_dim], fp8, kind="ExternalInput")
    hsT8_h = nc.dram_tensor("hsT8", [128, n_dt, bc], fp8, kind="ExternalInput")
    ab_h = nc.dram_tensor("ab_t", [128, n_dt], f32, kind="ExternalInput")
    v8_h = nc.dram_tensor("v8", [128, 2, n_dt], fp8, kind="ExternalInput")
    id_h = nc.dram_tensor("ident", [128, 128], bf16, kind="ExternalInput")
    out_h = nc.dram_tensor("out", [bc, e_dim], f32, kind="ExternalOutput")

    with tile.TileContext(nc) as tc, ExitStack() as ctx:
        consts = ctx.enter_context(tc.tile_pool(name="consts", bufs=1))
        nat_pool = ctx.enter_context(tc.tile_pool(name="nat", bufs=3 * n_chunks))
        eT_pool = ctx.enter_context(tc.tile_pool(name="eT", bufs=3 * n_chunks + 1))
        th_pool = ctx.enter_context(tc.tile_pool(name="th", bufs=2))
        sm_pool = ctx.enter_context(tc.tile_pool(name="softmax", bufs=3))
        small_pool = ctx.enter_context(tc.tile_pool(name="small", bufs=4))
        outsb_pool = ctx.enter_context(tc.tile_pool(name="outsb", bufs=1))
        pe_psum = ctx.enter_context(tc.tile_pool(name="pe_psum", bufs=2, space="PSUM"))
        sc_psum = ctx.enter_context(tc.tile_pool(name="sc_psum", bufs=2, space="PSUM"))
        w_psum = ctx.enter_context(tc.tile_pool(name="w_psum", bufs=1, space="PSUM"))

        # ---------------- consts ----------------
        ident_sb = consts.tile([128, 128], bf16)
        ones_bf = ident_sb[0:1, 0:1]
        ones_f = consts.tile([128, 1], f32)
        nc.vector.memset(ones_f, 1.0)

        w8_sb = consts.tile([128, n_g, 2, d_dim], fp8)
        # dual-fp8 ldweights needs a wide stride between the k-pair weight
        # blocks (walrus s3_lw_dual_fp8_restrictions rejects stride 2/4;
        # 512 verified on HW) -> stage v into a padded tile
        v8_sb = consts.tile([128, 2, 512], fp8)
        ab_sb = consts.tile([128, n_dt], f32)
        hsT8_sb = consts.tile([128, n_dt, bc], fp8)
        wd8_sb = consts.tile([128, n_dt, d_dim], fp8)

        def emit_consts(step):
            # wd8 lands before w8: the h_proj -> hb chain completes while
            # the PE is still waiting for w8 + the first transposes, so the
            # first tanh is never gated on hb
            if step == 0:
                nc.sync.dma_start(out=wd8_sb, in_=wd8_h[:, :, :])
                nc.sync.dma_start(out=hsT8_sb, in_=hsT8_h[:, :, :])
                nc.sync.dma_start(out=ab_sb, in_=ab_h[:, :])
            elif step == 1:
                nc.sync.dma_start(out=w8_sb[:, 0 : n_g // 2], in_=w8_h[:, 0 : n_g // 2])
                nc.sync.dma_start(out=w8_sb[:, n_g // 2 :], in_=w8_h[:, n_g // 2 :])
                nc.sync.dma_start(out=v8_sb[:, :, 0 : n_dt], in_=v8_h[:, :, :])
                nc.sync.dma_start(out=ident_sb, in_=id_h[:, :])

        hb_sb = consts.tile([128, n_dt, bc], f32)
        warm_sb = consts.tile([1, 1], f32)

        state = {}

        def emit_warmup(n_tr):
            # pull the activation-table load off the first-tanh critical path
            nc.scalar.activation(warm_sb, ones_f[0:1, 0:1], Tanh, bias=0.0, scale=1.0)
            # keep the PE busy until w8 lands so e_proj starts at full
            # p-state (the cost model halves PE speed for the first 3us
            # after an idle period)
            for k in range(n_tr):
                pswarm = sc_psum.tile([128, 128], f32, tag="sc")
                nc.tensor.matmul(
                    pswarm, lhsT=wd8_sb[:, 0:2, 0:128],
                    rhs=wd8_sb[:, 0:2, 0:128],
                    start=True, stop=True,
                    perf_mode=DR,
                )

        def emit_xbar_chunk(b, c):
            # transpose fp8 pairs (as u16 lanes) straight from DRAM, one
            # 512-row instruction per chunk:
            # out[p, g, q] = enc8u[b, CH*c + q, 128*g + p]
            eT = eT_pool.tile([128, n_g, CH], u16, tag="eT")
            nc.sync.dma_start(
                out=eT,
                in_=ench_h[b, CH * c : CH * (c + 1), :].bitcast(u16),
                transpose=True,
            )
            state[(b, c)] = dict(eT=eT)

        def emit_load_chunk(b, c):
            nath = nat_pool.tile([128, n_st, e_dim], fp8, tag="nath")
            nc.sync.dma_start(
                out=nath,
                in_=ench_h[b, CH * c : CH * (c + 1), :].rearrange(
                    "(j p) e -> p j e", p=128
                ),
            )
            natl = nat_pool.tile([128, n_st, e_dim], fp8, tag="natl")
            nc.sync.dma_start(
                out=natl,
                in_=encl_h[b, CH * c : CH * (c + 1), :].rearrange(
                    "(j p) e -> p j e", p=128
                ),
            )
            state[(b, c)]["nath"] = nath
            state[(b, c)]["natl"] = natl

        def emit_hproj():
            hp = sc_psum.tile([128, n_dt, bc], f32, tag="sc")
            for i in range(n_dt):
                for u in range(n_dt // 2):
                    nc.tensor.matmul(
                        hp[:, i, :],
                        lhsT=wd8_sb[:, 2 * u : 2 * u + 2, 128 * i : 128 * (i + 1)],
                        rhs=hsT8_sb[:, 2 * u : 2 * u + 2, :],
                        start=(u == 0),
                        stop=(u == n_dt // 2 - 1),
                        perf_mode=DR,
                    )
            for i in range(n_dt):
                nc.vector.tensor_scalar(
                    hb_sb[:, i, :], hp[:, i, :], 1.0 / 64,
                    ab_sb[:, i : i + 1],
                    op0=mybir.AluOpType.mult, op1=mybir.AluOpType.add,
                )

        def emit_eproj_pair(b, cp, mid_hook=None, pre_hook=None, tr_hook=None):
            eT = []
            for c in (2 * cp, 2 * cp + 1):
                # [p, g, q](u16) -> fp8 [p, g, (q two)]; per (j, g) the
                # DoubleRow rhs is [p, two, q]
                eT.append(state[(b, c)]["eT"][:, :, :].bitcast(fp8))
            th = th_pool.tile([128, n_dt, 2 * CH], fp8, tag="th")
            for i in range(n_dt):
                ps = pe_psum.tile([128, 2, CH], f32, tag="pe")
                for h in range(2):
                    for j in range(n_st):
                        for g in range(n_g):
                            rhs = eT[h][:, g, 256 * j : 256 * (j + 1)].rearrange(
                                "p (q two) -> p two q", two=2
                            )
                            nc.tensor.matmul(
                                ps[:, h, 128 * j : 128 * (j + 1)],
                                lhsT=w8_sb[:, g, :, 128 * i : 128 * (i + 1)],
                                rhs=rhs,
                                start=(g == 0),
                                stop=(g == n_g - 1),
                                perf_mode=DR,
                            )
                if mid_hook is not None:
                    # h_proj needs to land before the first tanh reads hb
                    # (program-order RAW), but after d-tile 0's matmuls so
                    # the PE ramps on e_proj while w_dec arrives
                    mid_hook()
                    mid_hook = None
                if pre_hook is not None:
                    # previous pair's scores/exp slot in here so the exp
                    # activations run before this pair's tanh queue on the
                    # Activation engine
                    pre_hook()
                    pre_hook = None
                if tr_hook is not None and i == 3:
                    # previous pair's attn transpose/mask/fp8-split: by
                    # d-tile 3 its exp has retired on ScalarE, and the DVE
                    # split chain overlaps d-tiles 4..7 here
                    tr_hook()
                    tr_hook = None
                nc.scalar.activation(
                    th[:, i, :], ps, Tanh, bias=hb_sb[:, i, b : b + 1], scale=1.0 / 64
                )
            state[(b, cp, "th")] = th

        def emit_scores(b, cp):
            th = state[(b, cp, "th")]
            scores = state[b]["scores"]
            for h in range(2):
                sc = sc_psum.tile([2, CH], f32, tag="sc")
                for m in range(n_dt // 2):
                    nc.tensor.matmul(
                        sc,
                        lhsT=v8_sb[:, :, 2 * m : 2 * m + 2],
                        rhs=th[:, 2 * m : 2 * m + 2, CH * h : CH * (h + 1)],
                        start=(m == 0),
                        stop=(m == n_dt // 2 - 1),
                        perf_mode=DR,
                    )
                pos = CH * (2 * cp + h)
                nc.scalar.activation(
                    scores[:, pos : pos + CH], sc[0:1, :], Exp, bias=0.0, scale=1.0 / 64
                )

        def emit_row_prep(b):
            maskT = sm_pool.tile([128, n_ct], bf16, tag="maskT")
            nc.sync.dma_start(out=maskT, in_=mskT_h[b, :, :])
            scores = sm_pool.tile([1, s], bf16, tag="scores")
            state[b] = dict(maskT=maskT, scores=scores)

        def emit_attn_prep(b, cp):
            # transpose this pair's exp(scores) into columns, apply the mask
            # during psum evacuation, and split attn into fp8 + residual.
            # Runs as a hook inside the NEXT pair's eproj so the DVE chain
            # overlaps PE matmuls instead of stalling the weighted sum.
            scores = state[b]["scores"]
            maskT = state[b]["maskT"]
            half = n_ct // 2
            j0 = cp * half
            psum_at = sc_psum.tile([128, half], f32, tag="sc")
            for j in range(half):
                nc.tensor.matmul(
                    psum_at[:, j : j + 1],
                    lhsT=scores[:, 128 * (j0 + j) : 128 * (j0 + j + 1)],
                    rhs=ones_bf,
                    start=True,
                    stop=True,
                )
            if cp == 0:
                attnT_new = small_pool.tile([128, n_ct], bf16, tag="attnT")
                ah_new = small_pool.tile([128, 2, 512], fp8, tag="ah")
                al_new = small_pool.tile([128, 2, 512], fp8, tag="al")
                pw_new = w_psum.tile([1, 2, 512], f32, tag="w")
                state[b]["attnT"] = attnT_new
                state[b]["ah"] = ah_new
                state[b]["al"] = al_new
                state[b]["pw"] = pw_new
            attnT = state[b]["attnT"]
            ah, al = state[b]["ah"], state[b]["al"]
            pw = state[b]["pw"]
            nc.vector.tensor_mul(
                attnT[:, j0 : j0 + half], psum_at, maskT[:, j0 : j0 + half]
            )
            # split attn into fp8 + fp8 residual, packed by s-tile-pair parity
            # with a 512 k-stride (dual-fp8 ldweights wants wide strides);
            # ah[p, i, u] = attn(tok=(2u+i)*128+p)
            u0 = half // 2 * cp
            nu = half // 2
            asrc = attnT[:, j0 : j0 + half].rearrange("p (u two) -> p two u", two=2)
            nc.vector.tensor_copy(out=ah[:, :, u0 : u0 + nu], in_=asrc)
            nc.vector.tensor_tensor(
                out=al[:, :, u0 : u0 + nu], in0=asrc, in1=ah[:, :, u0 : u0 + nu],
                op=mybir.AluOpType.subtract,
            )
            if cp == n_chunks // 2 - 1:
                # row sum + reciprocal ahead of the weighted matmuls: the
                # tiny sum matmul would otherwise queue behind them on the
                # PE, delaying the final evacuation by the whole pair
                partials = small_pool.tile([128, 1], f32, tag="part")
                nc.vector.reduce_sum(
                    out=partials, in_=attnT, axis=mybir.AxisListType.X
                )
                psum_s = sc_psum.tile([1, 1], f32, tag="sc")
                nc.tensor.matmul(
                    psum_s, lhsT=partials, rhs=ones_f, start=True, stop=True
                )
                rsum = small_pool.tile([1, 1], f32, tag="rsum")
                nc.vector.reciprocal(rsum, psum_s)
                state[b]["rsum"] = rsum
        def emit_weighted_mms(b, cp):
            ah, al = state[b]["ah"], state[b]["al"]
            pw = state[b]["pw"]
            # hi*hi + hi*lo + lo*hi accumulate into one psum group
            # (residuals are unscaled fp8, so no rescale is needed)
            first_u, last_u = n_ct // 4 * cp, n_ct // 4 * (cp + 1) - 1
            for ec in range(n_ec):
                for u in range(first_u, last_u + 1):
                    c, jj = divmod(2 * u, n_st)
                    nath = state[(b, c)]["nath"][:, jj : jj + 2, 512 * ec : 512 * (ec + 1)]
                    natl = state[(b, c)]["natl"][:, jj : jj + 2, 512 * ec : 512 * (ec + 1)]
                    for src_a, src_e, is_first, is_last in (
                        (ah, nath, cp == 0 and u == first_u, False),
                        (ah, natl, False, False),
                        (al, nath, False, cp == n_chunks // 2 - 1 and u == last_u),
                    ):
                        nc.tensor.matmul(
                            pw[:, ec, :],
                            lhsT=src_a[:, :, u : u + 1],
                            rhs=src_e,
                            start=is_first,
                            stop=is_last,
                            perf_mode=DR,
                        )

        def emit_weighted_finish(b):
            pw = state[b]["pw"]
            rsum = state[b]["rsum"]
            out_sb = outsb_pool.tile([1, e_dim], f32, tag="outsb")
            for ec in range(n_ec):
                nc.vector.tensor_scalar_mul(
                    out_sb[:, 512 * ec : 512 * (ec + 1)],
                    pw[:, ec, :],
                    rsum[0:1, 0:1],
                )
            nc.sync.dma_start(out=out_h[b : b + 1, :], in_=out_sb)

        # ---------------- schedule ----------------
        emit_xbar_chunk(0, 0)
        emit_xbar_chunk(0, 1)
        emit_consts(0)
        emit_consts(1)
        emit_hproj()
        emit_warmup(40)
        emit_xbar_chunk(0, 2)
        emit_xbar_chunk(0, 3)
        emit_row_prep(0)
        for c in range(n_chunks):
            emit_load_chunk(0, c)
        # one-pair stagger across the whole pipeline: while the PE runs
        # eproj of pair P, it then retires scores/exp/transpose/weighted of
        # pair P-1, whose Activation-side work completed during eproj(P) --
        # the PE never waits on ScalarE.
        pairs = [(b, cp) for b in range(bc) for cp in range(n_chunks // 2)]
        for idx, (b, cp) in enumerate(pairs):
            if idx == 0:
                emit_eproj_pair(b, cp)
            else:
                pb, pcp = pairs[idx - 1]
                emit_eproj_pair(
                    b, cp,
                    pre_hook=lambda pb=pb, pcp=pcp: emit_scores(pb, pcp),
                    tr_hook=lambda pb=pb, pcp=pcp: emit_attn_prep(pb, pcp),
                )
                emit_weighted_mms(pb, pcp)
                if pcp == n_chunks // 2 - 1:
                    emit_weighted_finish(pb)
            # loads come AFTER the retirement above (its weighted matmuls
            # free the nat ring slots these loads reuse), batched two rows
            # at a time so the copy<->transpose queue-mode switch drains
            # happen half as often
            if cp == 0:
                next_rows = [r for r in (
                    (b + 1, b + 2) if b % 2 == 0 else ()
                ) if r < bc]
                for r in next_rows:
                    for c in range(n_chunks):
                        emit_xbar_chunk(r, c)
                for r in next_rows:
                    emit_row_prep(r)
                    for c in range(n_chunks):
                        emit_load_chunk(r, c)
        pb, pcp = pairs[-1]
        emit_scores(pb, pcp)
        emit_attn_prep(pb, pcp)
        emit_weighted_mms(pb, pcp)
        emit_weighted_finish(pb)

    nc.compile()
    return nc


_CACHE = {}


def _prep_weights(a_w, a_b, v_w, e_dim=ENC, d_dim=DEC):
    import ml_dtypes

    fp8 = ml_dtypes.float8_e4m3
    n_g, n_dt = e_dim // 256, d_dim // 128
    # w8[p, g, i, d] = 64 * a_w[DEC + 2*(128*g + p) + i, d]
    w8 = (
        (np.asarray(a_w[d_dim:], np.float32) * 64.0)
        .reshape(n_g, 128, 2, d_dim).transpose(1, 0, 2, 3).astype(fp8)
    )
    wd8 = (
        (np.asarray(a_w[:d_dim], np.float32) * 64.0)
        .reshape(n_dt, 128, d_dim).transpose(1, 0, 2).astype(fp8)
    )
    # v8[p, i, 2m+r] = 64 * v_w[(2m+i)*128 + p]  (duplicated along r: the
    # dual-fp8 ldweights wants M=2 columns)
    v8 = np.repeat(
        (np.asarray(v_w, np.float32) * 64.0)
        .reshape(n_dt // 2, 2, 128).transpose(2, 1, 0).astype(fp8)[:, :, :, None],
        2, axis=3,
    ).reshape(128, 2, n_dt)
    ab_t = np.ascontiguousarray(
        np.asarray(a_b, np.float32).reshape(n_dt, 128).T
    )
    return (
        np.ascontiguousarray(w8),
        np.ascontiguousarray(wd8),
        np.ascontiguousarray(v8),
        ab_t,
    )


def kernel(hidden_states, encoder_outputs, encoder_masks, a_w, a_b, v_w):
    import ml_dtypes
    from concourse.bass_utils import run_bass_kernel_spmd

    if "nc" not in _CACHE:
        _CACHE["nc"] = build_bass_kernel()
    nc = _CACHE["nc"]

    bf16 = ml_dtypes.bfloat16
    fp8 = ml_dtypes.float8_e4m3
    hidden_states = np.asarray(hidden_states, dtype=np.float32)
    enc_f32 = np.asarray(encoder_outputs, dtype=np.float32)
    ench = enc_f32.astype(fp8)
    encl = (enc_f32 - ench.astype(np.float32)).astype(fp8)
    encoder_masks = np.asarray(encoder_masks, dtype=np.int32)
    w8, wd8, v8, ab_t = _prep_weights(a_w, a_b, v_w)
    ident = np.eye(128, dtype=bf16)
    n_dt = DEC // 128

    in_maps = []
    for c in range(N_CORES):
        sl = slice(c * BC, (c + 1) * BC)
        hsT8 = np.ascontiguousarray(
            hidden_states[sl].T.reshape(n_dt, 128, BC).transpose(1, 0, 2)
        ).astype(fp8)
        maskT = np.ascontiguousarray(
            (encoder_masks[sl] != 0)
            .reshape(BC, S // 128, 128).transpose(0, 2, 1)
        ).astype(bf16)
        m = {
            "ench": np.ascontiguousarray(ench[sl]),
            "encl": np.ascontiguousarray(encl[sl]),
            "maskT": maskT,
            "w8": w8,
            "wd8": wd8,
            "hsT8": np.ascontiguousarray(hsT8),
            "ab_t": ab_t,
            "v8": v8,
            "ident": ident,
        }
        in_maps.append(m)

    global _LAST_IN_MAPS
    _LAST_IN_MAPS = in_maps
    res = run_bass_kernel_spmd(nc, in_maps, core_ids=list(range(N_CORES)))
    out = np.concatenate([r["out"] for r in res.results], axis=0)
    return out.astype(np.float32)


_LAST_IN_MAPS = None
